# revision 1
# baseline (speedup 1.0000x reference)
"""CPC unsupervised criterion loss kernel for 8 Trainium2 NeuronCores.

Sharding: data-parallel over the nGt=8 batch axis, one sequence per core;
W and the otherEncoded negative pool replicated; the extIdx gather is local
per shard.

Per-core pipeline:
  - locC = (cFeature/256) @ W[k].T on PE in fp32, in two layouts: t-major
    (positive path) and e-major split into bf16 hi/lo planes (negative path).
  - The pool is host-split into bf16 hi/lo planes packed per row; the
    transposing dma_gather lands gathered rows e-major on partitions, which
    is exactly the matmul contraction layout (no on-chip transpose).
  - negScore = hi*hi + hi*lo + lo*hi bf16 matmuls accumulated in PSUM
    (bf16x3 ~ fp32 accuracy).
  - Scores are reordered k-major, PE-transposed to [(k,tau) x 128 negs], and
    reduced along the free dim: max (for accuracy) and sum(exp) (for the
    logsumexp; no max subtraction needed since |score| << 1).
  - Mean over t and the argmax==0 count via ones-vector matmuls.
"""

import os
import sys

import numpy as np

try:
    import concourse  # noqa: F401
except ImportError:
    sys.path.insert(0, "/opt/trn_rl_repo")

import ml_dtypes

import concourse.bacc as bacc
import concourse.bass as bass
import concourse.mybir as mybir
import concourse.tile as tile
from concourse import library_config
from concourse.bass_utils import run_bass_kernel_spmd

BF16NP = ml_dtypes.bfloat16
F32 = mybir.dt.float32
BF16 = mybir.dt.bfloat16
I16 = mybir.dt.int16

N_PREDICTS = 12
DIM = 256
NEG = 128
N_GT = 8
SEQ_LEN = 128
POOL = 8192
WIN = SEQ_LEN - N_PREDICTS  # 116

NCORES = 8
TG = 32                                  # t per group (32-aligned for slicing)
NGROUPS = 4
GCNT = [32, 32, 32, 20]                  # t per group; sums to 116
NA = 21                                  # taus in PSUM bank A (21*24=504 fp32)
ACT = mybir.ActivationFunctionType
ALU = mybir.AluOpType

_prog_cache = None


def _build_program():
    nc = bacc.Bacc("TRN2", target_bir_lowering=False, debug=False,
                   num_devices=NCORES, num_swdge_queues=2)

    poolhl = nc.declare_dram_parameter("poolhl", [POOL, 2 * DIM], BF16, isOutput=False)
    wt = nc.declare_dram_parameter("wt", [128, N_PREDICTS * 2 * DIM], F32, isOutput=False)
    ct = nc.declare_dram_parameter("ct", [128, 2 * WIN], F32, isOutput=False)
    gt = nc.declare_dram_parameter("gt", [128, DIM], F32, isOutput=False)
    idx = nc.declare_dram_parameter("idx", [128, NEG * WIN // 16], I16, isOutput=False)
    ones = nc.declare_dram_parameter("ones", [128, 1], F32, isOutput=False)
    ident = nc.declare_dram_parameter("ident", [128, 128], F32, isOutput=False)
    out = nc.declare_dram_parameter("out", [1, 2 * N_PREDICTS], F32, isOutput=True)

    with tile.TileContext(nc) as tc:
        with (
            tc.tile_pool(name="constp", bufs=1) as constp,
            tc.tile_pool(name="sbp", bufs=1) as sbp,
            tc.tile_pool(name="negp", bufs=17) as negp,
            tc.tile_pool(name="workp", bufs=2) as workp,
        ):
            # --- index load + first gathers as early as possible ---
            idxs = constp.tile([128, NEG * WIN // 16], I16)
            nc.sync.dma_start(idxs[:], idx[:])
            lib = nc.gpsimd.load_library(library_config.mlp)

            # The HW SWDGE descriptor carveout is 256 descs/engine and the
            # transposed gather needs n/4+2, capping one gather at 896 idxs.
            negts = []
            NSUB = 17                    # 7-t sub-gathers (896 idxs each)
            SUBCNT = [7] * 16 + [4]

            def emit_gather(i):
                n = SUBCNT[i] * NEG
                t = negp.tile([128, 4, n], BF16, tag="negT")
                gi = nc.gpsimd.dma_gather(
                    t[:], poolhl[:],
                    idxs[:, 56 * i:56 * i + n // 16],
                    n, n, 2 * DIM, transpose=True, queue_num=i % 2,
                )
                bass._add_dep_helper(gi.ins, lib.ins, sync=False,
                                     reason="gpsimd lib before gather")
                negts.append(t)

            for i in range(NSUB):
                emit_gather(i)

            # --- constant loads ---
            wtile = constp.tile([128, N_PREDICTS * 2 * DIM], F32)
            HW_ = N_PREDICTS * DIM
            nc.sync.dma_start(wtile[:, :HW_], wt[:, :HW_])
            nc.sync.dma_start(wtile[:, HW_:], wt[:, HW_:])
            ctile = constp.tile([128, 2 * WIN], F32)
            nc.sync.dma_start(ctile[:], ct[:])
            gtile = constp.tile([128, DIM], F32)
            nc.sync.dma_start(gtile[:], gt[:])
            onest = constp.tile([128, 1], F32)
            nc.sync.dma_start(onest[:], ones[:])
            identt = constp.tile([128, 128], F32)
            nc.sync.dma_start(identt[:], ident[:])

            # shifted copies of gt for the positive path:
            # gts[t, k*256+e] = gt[t+k+1, e]
            gts = constp.tile([128, N_PREDICTS * DIM], F32)
            for k in range(N_PREDICTS):
                nc.sync.dma_start(gts[:WIN, k * DIM:(k + 1) * DIM],
                                  gtile[k + 1:k + 1 + WIN, :])

            # --- locC in both layouts ---
            # lcg[c]: [128 (e of chunk c), t*24 + {0..11: hi(k), 12..23: lo(k)}]
            lcg = [constp.tile([128, WIN * 2 * N_PREDICTS], BF16,
                               tag=f"lcg{c}", name=f"lcg{c}")
                   for c in range(2)]
            posT = sbp.tile([WIN, N_PREDICTS], F32)

            with tc.tile_pool(name="ps_lc", bufs=2, space="PSUM") as ps_lc:
                for mc in range(2):
                    for k in range(N_PREDICTS):
                        p1 = ps_lc.tile([128, WIN], F32, tag="l1")
                        for dc in range(2):
                            nc.tensor.matmul(
                                p1[:, :],
                                wtile[:, (k * 2 + dc) * DIM + mc * 128:
                                      (k * 2 + dc) * DIM + mc * 128 + 128],
                                ctile[:, dc * WIN:(dc + 1) * WIN],
                                start=(dc == 0), stop=(dc == 1),
                            )
                        hi_ap = lcg[mc][:].rearrange("p (t x) -> p t x", x=24)[:, :, k]
                        lo_ap = lcg[mc][:].rearrange("p (t x) -> p t x", x=24)[:, :, 12 + k]
                        nc.scalar.activation(hi_ap, p1[:, :], ACT.Copy)
                        nc.vector.tensor_tensor(out=lo_ap, in0=p1[:, :], in1=hi_ap,
                                                op=ALU.subtract)

            pools2 = (
                tc.tile_pool(name="ps_sc", bufs=2, space="PSUM"),
                tc.tile_pool(name="ps_tr", bufs=2, space="PSUM"),
                tc.tile_pool(name="ps_fin", bufs=1, space="PSUM"),
            )
            with pools2[0] as ps_sc, pools2[1] as ps_tr, pools2[2] as ps_fin:
                pos32 = sbp.tile([TG, NGROUPS * N_PREDICTS], F32)
                nc.vector.memset(pos32[:, 36:48], 0.0)

                Mt = sbp.tile([TG, NGROUPS * N_PREDICTS], F32)
                St = sbp.tile([TG, NGROUPS * N_PREDICTS], F32)
                nc.vector.memset(Mt[:, 36:48], 0.0)
                nc.vector.memset(St[:, 36:48], 1.0)
                Mt128 = sbp.tile([128, N_PREDICTS], F32)   # rows kap*32+tau
                St128 = sbp.tile([128, N_PREDICTS], F32)

                # --- main loop over groups of up to 32 t ---
                NHALF = 16

                def reduce_group(g, ssb):
                    for j in range(3):
                        trp = ps_tr.tile([128, 128], F32, tag="trp", name="trp")
                        nc.tensor.transpose(trp[:, :],
                                            ssb[:, j * 128:(j + 1) * 128],
                                            identt[:, :])
                        c2 = g * 3 + j
                        nc.vector.tensor_reduce(out=Mt128[:, c2:c2 + 1],
                                                in_=trp[:, :],
                                                axis=mybir.AxisListType.X,
                                                op=ALU.max)
                        esc = workp.tile([128, 128], F32, tag="esc", name="esc")
                        nc.scalar.activation(esc[:, :], trp[:, :], ACT.Exp,
                                             accum_out=St128[:, c2:c2 + 1])
                    # bridge this group's three columns r-layout -> tau-layout
                    rows = GCNT[g]
                    for kap in range(4):
                        for T128, T32 in ((Mt128, Mt), (St128, St)):
                            nc.sync.dma_start(
                                T32[:rows].rearrange("p (m x) -> p m x", x=4)
                                [:, 3 * g:3 * g + 3, kap],
                                T128[kap * TG:kap * TG + rows,
                                     3 * g:3 * g + 3])

                pending = []
                for g in range(NGROUPS):
                    cnt = GCNT[g]
                    # two PSUM tiles per group: 24 columns per tau
                    # (12 hi-products | 12 lo-products), summed on copy-out
                    psh = [ps_sc.tile([128, NHALF * 24], F32, tag=f"ps{h}",
                                      name=f"ps{h}") for h in range(2)]
                    for tau in range(cnt):
                        t = TG * g + tau
                        s, off = (t // 7, t % 7) if t < 112 else (16, t - 112)
                        negT = negts[s]
                        ps = psh[tau // NHALF]
                        lo = tau % NHALF
                        ms24 = ps[:, lo * 24:lo * 24 + 24]
                        ms12 = ps[:, lo * 24:lo * 24 + 12]
                        sl = slice(off * 128, off * 128 + 128)
                        hilo0 = lcg[0][:, t * 24:t * 24 + 24]
                        hi0 = lcg[0][:, t * 24:t * 24 + 12]
                        hilo1 = lcg[1][:, t * 24:t * 24 + 24]
                        hi1 = lcg[1][:, t * 24:t * 24 + 12]
                        nc.tensor.matmul(ms24, negT[:, 0, sl], hilo0,
                                         start=True, stop=False)
                        nc.tensor.matmul(ms12, negT[:, 2, sl], hi0,
                                         start=False, stop=False)
                        nc.tensor.matmul(ms12, negT[:, 3, sl], hi1,
                                         start=False, stop=False)
                        nc.tensor.matmul(ms24, negT[:, 1, sl], hilo1,
                                         start=False, stop=True)

                    # (tau,k)->(k,tau) reorder: ssb[p, k*32+tau] = hi + lo
                    ssb = workp.tile([128, N_PREDICTS * TG], F32, tag="ssb", bufs=4)
                    o_ap = ssb[:].rearrange("p (k t) -> p t k", k=N_PREDICTS)
                    for h in range(2):
                        n0 = h * NHALF
                        nh = min(cnt - n0, NHALF)
                        if nh <= 0:
                            break
                        ip = psh[h][:].rearrange("p (t x) -> p t x", x=24)
                        osl = o_ap[:, n0:n0 + nh, :]
                        nc.vector.tensor_copy(osl, ip[:, 0:nh, 0:12])
                        nc.vector.tensor_tensor(out=osl, in0=ip[:, 0:nh, 12:24],
                                                in1=osl, op=ALU.add)

                    # lag the PE transposes by one group to keep score
                    # matmuls dense on the PE queue
                    pending.append((g, ssb))
                    if len(pending) > 1:
                        reduce_group(*pending.pop(0))
                for item in pending:
                    reduce_group(*item)

                # --- positive path (off the critical startup path) ---
                for k in range(N_PREDICTS):
                    p2 = ps_fin.tile([WIN, DIM], F32, tag="l2", name="l2")
                    for dc in range(2):
                        nc.tensor.matmul(
                            p2[:, :],
                            ctile[:, dc * WIN:(dc + 1) * WIN],
                            wtile[:, (k * 2 + dc) * DIM:(k * 2 + dc + 1) * DIM],
                            start=(dc == 0), stop=(dc == 1),
                        )
                    scr = workp.tile([WIN, DIM], F32, tag="scr", name="scr")
                    nc.vector.tensor_tensor(out=scr[:, :], in0=p2[:, :],
                                            in1=gts[:WIN, k * DIM:(k + 1) * DIM],
                                            op=ALU.mult)
                    nc.vector.tensor_reduce(out=posT[:, k:k + 1], in_=scr[:, :],
                                            axis=mybir.AxisListType.X, op=ALU.add)
                # pos32[tau, g*12+k] = posScore[k, 32g+tau] (partition shifts)
                for g in range(NGROUPS):
                    nc.sync.dma_start(pos32[:GCNT[g], g * 12:(g + 1) * 12],
                                      posT[TG * g:TG * g + GCNT[g], :])

                # --- combine ---
                expP = sbp.tile([TG, NGROUPS * N_PREDICTS], F32)
                nc.scalar.activation(expP[:, :], pos32[:, :], ACT.Exp)
                tmp1 = sbp.tile([TG, NGROUPS * N_PREDICTS], F32)
                nc.vector.tensor_tensor(out=tmp1[:, :], in0=expP[:, :], in1=St[:, :],
                                        op=ALU.add)
                tmp2 = sbp.tile([TG, NGROUPS * N_PREDICTS], F32)
                nc.scalar.activation(tmp2[:, :], tmp1[:, :], ACT.Ln)
                comb = sbp.tile([TG, NGROUPS * N_PREDICTS * 2], F32)
                c_ap = comb[:].rearrange("p (g k q) -> p q g k", q=2, g=NGROUPS)
                t2 = tmp2[:].rearrange("p (g k) -> p g k", g=NGROUPS)
                p2_ = pos32[:].rearrange("p (g k) -> p g k", g=NGROUPS)
                m2 = Mt[:].rearrange("p (g k) -> p g k", g=NGROUPS)
                nc.vector.tensor_tensor(out=c_ap[:, 0], in0=t2, in1=p2_, op=ALU.subtract)
                nc.vector.tensor_tensor(out=c_ap[:, 1], in0=p2_, in1=m2, op=ALU.is_ge)

                fin = ps_fin.tile([1, 2 * N_PREDICTS], F32, tag="fin")
                for g in range(NGROUPS):
                    nc.tensor.matmul(fin[:, :], onest[:GCNT[g], :],
                                     comb[:GCNT[g], g * 24:(g + 1) * 24],
                                     start=(g == 0), stop=(g == NGROUPS - 1))
                outsb = sbp.tile([1, 2 * N_PREDICTS], F32)
                f_ap = fin[:].rearrange("p (k q) -> p q k", q=2)
                os_ap = outsb[:].rearrange("p (k q) -> p q k", q=2)
                nc.scalar.activation(os_ap[:, 0], f_ap[:, 0], ACT.Copy, scale=1.0 / WIN)
                nc.scalar.activation(os_ap[:, 1], f_ap[:, 1], ACT.Copy,
                                     scale=1.0 / (N_GT * WIN))
                nc.sync.dma_start(out[:], outsb[:])

    nc.compile()
    return nc


def _host_prep(cFeature, gtPredictions, otherEncoded, W, extIdx):
    """Build the 8 per-core input maps."""
    pool_f32 = np.asarray(otherEncoded, dtype=np.float32)
    hi = pool_f32.astype(BF16NP)
    lo = (pool_f32 - hi.astype(np.float32)).astype(BF16NP)
    poolhl = np.ascontiguousarray(np.concatenate([hi, lo], axis=1))  # [8192, 512]

    W = np.asarray(W, dtype=np.float32)
    # wt[p, (k*2+c)*256 + e] = W[k, e, 128c+p]
    wt_np = np.ascontiguousarray(
        W.transpose(0, 2, 1).reshape(N_PREDICTS, 2, 128, DIM)
        .transpose(2, 0, 1, 3).reshape(128, N_PREDICTS * 2 * DIM))

    ones_np = np.ones((128, 1), dtype=np.float32)
    ident_np = np.eye(128, dtype=np.float32)

    ext = np.asarray(extIdx).reshape(N_GT, NEG, WIN)

    in_maps = []
    for b in range(N_GT):
        cb = np.asarray(cFeature[b, :WIN], dtype=np.float32) / DIM  # [116, 256]
        # ct[p, c*116+t] = cb[t, 128c+p]
        ct_np = np.ascontiguousarray(
            cb.T.reshape(2, 128, WIN).transpose(1, 0, 2).reshape(128, 2 * WIN))
        gt_np = np.ascontiguousarray(np.asarray(gtPredictions[b], dtype=np.float32))
        flat = np.ascontiguousarray(ext[b].T).reshape(-1)  # i = t*128 + n
        idx_np = np.ascontiguousarray(
            np.tile(flat.reshape(-1, 16).T, (8, 1))).astype(np.int16)
        in_maps.append({
            "poolhl": poolhl,
            "wt": wt_np,
            "ct": ct_np,
            "gt": gt_np,
            "idx": idx_np,
            "ones": ones_np,
            "ident": ident_np,
        })
    return in_maps


def kernel(cFeature, gtPredictions, otherEncoded, W, extIdx):
    global _prog_cache
    if _prog_cache is None:
        _prog_cache = _build_program()
    nc = _prog_cache
    in_maps = _host_prep(cFeature, gtPredictions, otherEncoded, W, extIdx)
    res = run_bass_kernel_spmd(nc, in_maps, list(range(NCORES)))
    losses = np.zeros(N_PREDICTS * N_GT, dtype=np.float32)
    acc = np.zeros(N_PREDICTS * N_GT, dtype=np.float32)
    for b in range(N_GT):
        o = res.results[b]["out"].reshape(N_PREDICTS, 2)
        losses[np.arange(N_PREDICTS) * N_GT + b] = o[:, 0]
        acc[np.arange(N_PREDICTS) * N_GT + b] = o[:, 1]
    return losses, acc


if __name__ == "__main__":
    sys.path.insert(0, os.path.dirname(os.path.abspath(__file__)))
    import reference

    inputs = reference.setup_inputs()
    inputs = {k: np.asarray(v) for k, v in inputs.items()}
    got_losses, got_acc = kernel(**inputs)
    print("losses:", got_losses[:8])
    print("acc:", got_acc[:8])

